# revision 1
# baseline (speedup 1.0000x reference)
"""Grouped GEMM (MoE block-diagonal) on 8 Trainium2 NeuronCores.

Problem: x [262144, 256] bf16, w [1024, 256] bf16 (G=8 experts of [128, 256]).
Rows g*32768:(g+1)*32768 of x belong to expert g.
Output [262144, 1024] bf16, block-diagonal: out[rows_g, g*128:(g+1)*128] = x_g @ w_g^T.

Strategy (expert-parallel):
  - Core g gets expert g: x_g [32768, 256] and w_g [128, 256].
  - Host pre-transposes both operands so the contraction dim K lands on SBUF
    partitions (PE matmul contracts over the partition dim) — no on-device
    transpose needed.
  - Device computes yT_g [128, 32768] = w_g @ x_g^T with lhsT = w_g^T
    (stationary) and rhs = x_g^T (moving, 512-token PSUM tiles), so every
    DMA (in and out) moves multi-KB contiguous runs per partition.
  - Host transposes yT_g back and scatters into the zero-filled
    block-diagonal output (the zero blocks never touch the device).
"""

import sys

for _p in ("/opt/trn_rl_repo", "/root/.axon_site/_ro/trn_rl_repo"):
    if _p not in sys.path:
        sys.path.insert(0, _p)

import numpy as np

G = 8          # experts == cores
K = 256        # contraction dim
N = 128        # output dim per expert
M = 262144     # total tokens
MPC = M // G   # tokens per core = 32768

MT = 8192      # tokens per outer tile (SBUF staging)
PT = 512       # tokens per matmul (max PE free dim)
PB = 1024      # tokens per PSUM tile (2 banks, 2 matmul-pairs)
KP = 128       # partition chunk of K


def _split_multi_waits(nc, mybir):
    """This walrus build rejects any instruction carrying more than one sync
    wait ("Too many sync wait commands", setupSyncWait). Hoist all but one
    wait of each offender onto fresh single-wait EventSemaphore instructions
    placed just before it on the same engine queue — semantically identical
    (sequencer-level blocking, monotonic sem conditions)."""
    for fn in nc.m.functions:
        for blk in fn.blocks:
            new_insts = []
            for inst in blk.instructions:
                si = getattr(inst, "sync_info", None)
                waits = list(si.on_wait) if si is not None and si.on_wait else []
                if len(waits) > 1:
                    for w in waits[:-1]:
                        name = nc.get_next_instruction_name()
                        ev = mybir.InstEventSemaphore(
                            name=name,
                            engine=inst.engine,
                            ins=[],
                            outs=[],
                            sync_info=mybir.SyncInfo(on_wait=[w], on_update=[]),
                        )
                        nc.inst_map[name] = ev
                        new_insts.append(ev)
                    si.on_wait = waits[-1:]
                new_insts.append(inst)
            blk.instructions = new_insts


def _build_bass():
    import concourse.bass as bass
    import concourse.mybir as mybir
    import concourse.tile as tile

    bf16 = mybir.dt.bfloat16
    f32 = mybir.dt.float32

    nc = bass.Bass()
    xT = nc.declare_dram_parameter("xT", [K, MPC], bf16, isOutput=False)
    wT = nc.declare_dram_parameter("wT", [K, N], bf16, isOutput=False)
    yT = nc.declare_dram_parameter("yT", [N, MPC], bf16, isOutput=True)

    with tile.TileContext(nc) as tc:
        with (
            tc.tile_pool(name="w", bufs=1) as wpool,
            tc.tile_pool(name="x", bufs=2) as xpool,
            tc.tile_pool(name="y", bufs=2) as ypool,
            tc.tile_pool(name="ps", bufs=4, space=bass.MemorySpace.PSUM) as pspool,
        ):
            w_t = wpool.tile([KP, 2, N], bf16)
            nc.sync.dma_start(
                w_t[:], wT[:, :].rearrange("(two p) n -> p two n", two=2)
            )

            LCH = 2048  # tokens per x-load chunk
            ST = 4096   # tokens per y-store chunk
            for mo in range(0, MPC, MT):
                x_t = xpool.tile([KP, 2, MT], bf16)
                # Chunked loads: compute on chunk c starts as soon as chunk c
                # lands, instead of gating on the full tile.
                for off in range(0, MT, LCH):
                    nc.sync.dma_start(
                        x_t[:, :, off : off + LCH],
                        xT[:, mo + off : mo + off + LCH].rearrange(
                            "(two p) m -> p two m", two=2
                        ),
                    )

                y_t = ypool.tile([N, MT], bf16)
                for i, mb in enumerate(range(0, MT, PB)):
                    # One PSUM tile spans 2 banks; each 512-token half gets
                    # its own accumulation group, then one double-width cast.
                    ps = pspool.tile([N, PB], f32)
                    for ms in (mb, mb + PT):
                        o = ms - mb
                        nc.tensor.matmul(
                            ps[:, o : o + PT],
                            w_t[:, 0, :],
                            x_t[:, 0, ms : ms + PT],
                            start=True,
                            stop=False,
                        )
                        nc.tensor.matmul(
                            ps[:, o : o + PT],
                            w_t[:, 1, :],
                            x_t[:, 1, ms : ms + PT],
                            start=False,
                            stop=True,
                        )
                    if i % 3 == 2:
                        nc.scalar.copy(y_t[:, mb : mb + PB], ps[:])
                    else:
                        nc.vector.tensor_copy(y_t[:, mb : mb + PB], ps[:])
                    # Store each finished chunk as soon as its copies land.
                    if (mb + PB) % ST == 0:
                        so = mb + PB - ST
                        nc.scalar.dma_start(
                            yT[:, mo + so : mo + so + ST], y_t[:, so : so + ST]
                        )

    _split_multi_waits(nc, mybir)
    return nc


_NC_CACHE = None


def _get_nc():
    global _NC_CACHE
    if _NC_CACHE is None:
        _NC_CACHE = _build_bass()
    return _NC_CACHE


def _run(in_maps, **kwargs):
    from concourse.bass_utils import run_bass_kernel_spmd

    return run_bass_kernel_spmd(_get_nc(), in_maps, list(range(G)), **kwargs)


def make_in_maps(x, w):
    x = np.asarray(x)
    w = np.asarray(w)
    in_maps = []
    for g in range(G):
        xg = x[g * MPC : (g + 1) * MPC, :]
        wg = w[g * N : (g + 1) * N, :]
        in_maps.append(
            {
                "xT": np.ascontiguousarray(xg.T),
                "wT": np.ascontiguousarray(wg.T),
            }
        )
    return in_maps


def assemble(results, dtype):
    out = np.zeros((M, G * N), dtype=dtype)
    for g in range(G):
        yTg = np.asarray(results[g]["yT"])
        out[g * MPC : (g + 1) * MPC, g * N : (g + 1) * N] = yTg.T
    return out


def kernel(x, w):
    x = np.asarray(x)
    w = np.asarray(w)
    res = _run(make_in_maps(x, w))
    return assemble(res.results, x.dtype)



# revision 2
# speedup vs baseline: 1.0834x; 1.0834x over previous
"""Grouped GEMM (MoE block-diagonal) on 8 Trainium2 NeuronCores.

Problem: x [262144, 256] bf16, w [1024, 256] bf16 (G=8 experts of [128, 256]).
Rows g*32768:(g+1)*32768 of x belong to expert g.
Output [262144, 1024] bf16, block-diagonal: out[rows_g, g*128:(g+1)*128] = x_g @ w_g^T.

Strategy (expert-parallel):
  - Core g gets expert g: x_g [32768, 256] and w_g [128, 256].
  - Host packs both operands so the contraction dim K lands on SBUF
    partitions (PE matmul contracts over the partition dim) AND every load
    DMA moves one 16 KiB contiguous run per partition: xP[p, c*8192 + h*4096
    + t] = x_g^T[h*128+p, c*4096+t].  4 KiB runs cap the HWDGE load stream
    at ~240 GB/s (73 ns/packet fixed cost); 16 KiB runs let the 16 SDMA
    engines reach the HBM roofline.
  - Device computes yT_g [128, 32768] = w_g @ x_g^T with lhsT = w_g^T
    (stationary, both K-halves resident) and rhs = packed x columns,
    512-token matmuls accumulating K over 2 halves into [128,1024] PSUM.
  - Stores taper at the end (4096/2048/1024/1024 tokens) so the final
    store's flight time after the last cast is short.
  - Host transposes yT_g back and scatters into the zero-filled
    block-diagonal output (the zero blocks never touch the device).
"""

import sys

for _p in ("/opt/trn_rl_repo", "/root/.axon_site/_ro/trn_rl_repo"):
    if _p not in sys.path:
        sys.path.insert(0, _p)

import numpy as np

G = 8          # experts == cores
K = 256        # contraction dim
N = 128        # output dim per expert
M = 262144     # total tokens
MPC = M // G   # tokens per core = 32768

LCH = 4096     # tokens per load chunk (2 MB, 16 KiB/partition contiguous)
MT = 8192      # tokens per tile (2 chunks)
PT = 512       # tokens per matmul (max PE free dim)
PB = 1024      # tokens per PSUM tile


def _split_multi_waits(nc, mybir):
    """This walrus build rejects any instruction carrying more than one sync
    wait ("Too many sync wait commands", setupSyncWait). Hoist all but one
    wait of each offender onto fresh single-wait EventSemaphore instructions
    placed just before it on the same engine queue — semantically identical
    (sequencer-level blocking, monotonic sem conditions)."""
    for fn in nc.m.functions:
        for blk in fn.blocks:
            new_insts = []
            for inst in blk.instructions:
                si = getattr(inst, "sync_info", None)
                waits = list(si.on_wait) if si is not None and si.on_wait else []
                if len(waits) > 1:
                    for w in waits[:-1]:
                        name = nc.get_next_instruction_name()
                        ev = mybir.InstEventSemaphore(
                            name=name,
                            engine=inst.engine,
                            ins=[],
                            outs=[],
                            sync_info=mybir.SyncInfo(on_wait=[w], on_update=[]),
                        )
                        nc.inst_map[name] = ev
                        new_insts.append(ev)
                    si.on_wait = waits[-1:]
                new_insts.append(inst)
            blk.instructions = new_insts


def _build_bass():
    import concourse.bass as bass
    import concourse.mybir as mybir
    import concourse.tile as tile

    bf16 = mybir.dt.bfloat16
    f32 = mybir.dt.float32

    nc = bass.Bass()
    xP = nc.declare_dram_parameter("xP", [N, 2 * MPC], bf16, isOutput=False)
    wP = nc.declare_dram_parameter("wP", [N, K], bf16, isOutput=False)
    yT = nc.declare_dram_parameter("yT", [N, MPC], bf16, isOutput=True)

    with tile.TileContext(nc) as tc:
        with (
            tc.tile_pool(name="w", bufs=1) as wpool,
            tc.tile_pool(name="x", bufs=3) as xpool,
            tc.tile_pool(name="y", bufs=2) as ypool,
            tc.tile_pool(name="ps", bufs=4, space=bass.MemorySpace.PSUM) as pspool,
        ):
            # w on the scalar (ACT) HWDGE ring so the sync ring's first
            # descriptor is the first x chunk.
            w_t = wpool.tile([N, K], bf16)
            nc.scalar.dma_start(w_t[:], wP[:, :])

            last_mo = MPC - MT
            for mo in range(0, MPC, MT):
                x_t = xpool.tile([N, 2 * MT], bf16)
                # One DMA per 4096-token chunk: 16 KiB contiguous per
                # partition on both sides.
                for s in range(MT // LCH):
                    c = (mo + s * LCH) // LCH
                    nc.sync.dma_start(
                        x_t[:, s * 2 * LCH : (s + 1) * 2 * LCH],
                        xP[:, c * 2 * LCH : (c + 1) * 2 * LCH],
                    )

                y_t = ypool.tile([N, MT], bf16)
                # Store boundaries (tokens within the tile, exclusive ends).
                if mo == last_mo:
                    stores = [4096, 6144, 7168, 8192]
                else:
                    stores = [4096, 8192]
                prev_store = 0

                for i, mb in enumerate(range(0, MT, PB)):
                    ps = pspool.tile([N, PB], f32)
                    s = mb // LCH
                    off = mb % LCH
                    for o in (0, PT):
                        col = s * 2 * LCH + off + o
                        nc.tensor.matmul(
                            ps[:, o : o + PT],
                            w_t[:, 0:N],
                            x_t[:, col : col + PT],
                            start=True,
                            stop=False,
                        )
                        nc.tensor.matmul(
                            ps[:, o : o + PT],
                            w_t[:, N : 2 * N],
                            x_t[:, col + LCH : col + LCH + PT],
                            start=False,
                            stop=True,
                        )
                    if i % 3 == 2:
                        nc.scalar.copy(y_t[:, mb : mb + PB], ps[:])
                    else:
                        nc.vector.tensor_copy(y_t[:, mb : mb + PB], ps[:])
                    if mb + PB in stores:
                        so, se = prev_store, mb + PB
                        prev_store = se
                        nc.scalar.dma_start(
                            yT[:, mo + so : mo + se], y_t[:, so:se]
                        )

    _split_multi_waits(nc, mybir)
    return nc


_NC_CACHE = None


def _get_nc():
    global _NC_CACHE
    if _NC_CACHE is None:
        _NC_CACHE = _build_bass()
    return _NC_CACHE


def _run(in_maps, **kwargs):
    from concourse.bass_utils import run_bass_kernel_spmd

    return run_bass_kernel_spmd(_get_nc(), in_maps, list(range(G)), **kwargs)


def make_in_maps(x, w):
    x = np.asarray(x)
    w = np.asarray(w)
    in_maps = []
    for g in range(G):
        xg = x[g * MPC : (g + 1) * MPC, :]
        wg = w[g * N : (g + 1) * N, :]
        # xP[p, c*8192 + h*4096 + t] = xg.T[h*128+p, c*4096+t]
        xPg = np.ascontiguousarray(
            xg.T.reshape(2, N, MPC // LCH, LCH)
            .transpose(1, 2, 0, 3)
            .reshape(N, 2 * MPC)
        )
        # wP[p, h*128+n] = wg.T[h*128+p, n]
        wPg = np.ascontiguousarray(
            wg.T.reshape(2, N, N).transpose(1, 0, 2).reshape(N, K)
        )
        in_maps.append({"xP": xPg, "wP": wPg})
    return in_maps


def assemble(results, dtype):
    out = np.zeros((M, G * N), dtype=dtype)
    for g in range(G):
        yTg = np.asarray(results[g]["yT"])
        out[g * MPC : (g + 1) * MPC, g * N : (g + 1) * N] = yTg.T
    return out


def kernel(x, w):
    x = np.asarray(x)
    w = np.asarray(w)
    res = _run(make_in_maps(x, w))
    return assemble(res.results, x.dtype)


# revision 6
# speedup vs baseline: 1.1106x; 1.0251x over previous
"""Grouped GEMM (MoE block-diagonal) on 8 Trainium2 NeuronCores.

Problem: x [262144, 256] bf16, w [1024, 256] bf16 (G=8 experts of [128, 256]).
Rows g*32768:(g+1)*32768 of x belong to expert g.
Output [262144, 1024] bf16, block-diagonal: out[rows_g, g*128:(g+1)*128] = x_g @ w_g^T.

Strategy (expert-parallel):
  - Core g gets expert g: x_g [32768, 256] and w_g [128, 256].
  - Host packs both operands so the contraction dim K lands on SBUF
    partitions (PE matmul contracts over the partition dim) AND every load
    DMA moves one 16 KiB contiguous run per partition: xP[p, c*8192 + h*4096
    + t] = x_g^T[h*128+p, c*4096+t].  4 KiB runs cap the HWDGE load stream
    at ~240 GB/s (73 ns/packet fixed cost); 16 KiB runs let the 16 SDMA
    engines reach the HBM roofline.
  - Device computes yT_g [128, 32768] = w_g @ x_g^T with lhsT = w_g^T
    (stationary, both K-halves resident) and rhs = packed x columns,
    512-token matmuls accumulating K over 2 halves into [128,1024] PSUM.
  - Stores taper at the end (4096/2048/1024/1024 tokens) so the final
    store's flight time after the last cast is short.
  - Host transposes yT_g back and scatters into the zero-filled
    block-diagonal output (the zero blocks never touch the device).
"""

import sys

for _p in ("/opt/trn_rl_repo", "/root/.axon_site/_ro/trn_rl_repo"):
    if _p not in sys.path:
        sys.path.insert(0, _p)

import numpy as np

G = 8          # experts == cores
K = 256        # contraction dim
N = 128        # output dim per expert
M = 262144     # total tokens
MPC = M // G   # tokens per core = 32768

MT = 8192      # tokens per tile
PT = 512       # tokens per matmul (max PE free dim)
PB = 1024      # tokens per PSUM tile

# Load-chunk schedule per tile (token counts, each a PB multiple). The bulk
# uses 4096-token chunks (16 KiB contiguous per partition -> near-line-rate
# descriptors); the final tile tapers so the last chunk's compute+store tail
# after the load stream ends is short.
TILE_CHUNKS = [
    [4096, 4096],
    [4096, 4096],
    [4096, 4096],
    [4096, 2048, 1024, 1024],
]
# Store boundaries per tile (exclusive token ends within the tile); the last
# tile tapers so the final store is small and lands right after its cast.
TILE_STORES = [
    [4096, 8192],
    [4096, 8192],
    [4096, 8192],
    [4096, 6144, 7168, 8192],
]


def _split_multi_waits(nc, mybir):
    """This walrus build rejects any instruction carrying more than one sync
    wait ("Too many sync wait commands", setupSyncWait). Hoist all but one
    wait of each offender onto fresh single-wait EventSemaphore instructions
    placed just before it on the same engine queue — semantically identical
    (sequencer-level blocking, monotonic sem conditions)."""
    for fn in nc.m.functions:
        for blk in fn.blocks:
            new_insts = []
            for inst in blk.instructions:
                si = getattr(inst, "sync_info", None)
                waits = list(si.on_wait) if si is not None and si.on_wait else []
                if len(waits) > 1:
                    for w in waits[:-1]:
                        name = nc.get_next_instruction_name()
                        ev = mybir.InstEventSemaphore(
                            name=name,
                            engine=inst.engine,
                            ins=[],
                            outs=[],
                            sync_info=mybir.SyncInfo(on_wait=[w], on_update=[]),
                        )
                        nc.inst_map[name] = ev
                        new_insts.append(ev)
                    si.on_wait = waits[-1:]
                new_insts.append(inst)
            blk.instructions = new_insts


def _build_bass():
    import concourse.bass as bass
    import concourse.mybir as mybir
    import concourse.tile as tile

    bf16 = mybir.dt.bfloat16
    f32 = mybir.dt.float32

    nc = bass.Bass()
    xP = nc.declare_dram_parameter("xP", [N, 2 * MPC], bf16, isOutput=False)
    wP = nc.declare_dram_parameter("wP", [N, K], bf16, isOutput=False)
    yT = nc.declare_dram_parameter("yT", [N, MPC], bf16, isOutput=True)

    with tile.TileContext(nc) as tc:
        with (
            tc.tile_pool(name="w", bufs=1) as wpool,
            tc.tile_pool(name="x", bufs=4) as xpool,
            tc.tile_pool(name="y", bufs=2) as ypool,
            tc.tile_pool(name="ps", bufs=4, space=bass.MemorySpace.PSUM) as pspool,
        ):
            # w on the scalar (ACT) HWDGE ring so the sync ring's first
            # descriptor is the first x chunk.
            w_t = wpool.tile([N, K], bf16)
            nc.scalar.dma_start(w_t[:], wP[:, :])

            xcol = 0  # running column offset into xP (2 cols per token)
            for t, chunks in enumerate(TILE_CHUNKS):
                mo = t * MT
                x_t = xpool.tile([N, 2 * MT], bf16)
                # One DMA per chunk; each is one contiguous run per
                # partition on both sides (2*L tokens * 2 B).
                cbase = []  # (tile-token base, SBUF column base, L)
                tcol = 0
                tbase = 0
                for L in chunks:
                    nc.sync.dma_start(
                        x_t[:, tcol : tcol + 2 * L],
                        xP[:, xcol : xcol + 2 * L],
                    )
                    cbase.append((tbase, tcol, L))
                    tbase += L
                    tcol += 2 * L
                    xcol += 2 * L

                y_t = ypool.tile([N, MT], bf16)
                stores = TILE_STORES[t]
                prev_store = 0
                ci = 0

                for i, mb in enumerate(range(0, MT, PB)):
                    while mb >= cbase[ci][0] + cbase[ci][2]:
                        ci += 1
                    tb, tc, L = cbase[ci]
                    ps = pspool.tile([N, PB], f32)
                    for o in (0, PT):
                        col = tc + (mb - tb) + o
                        nc.tensor.matmul(
                            ps[:, o : o + PT],
                            w_t[:, 0:N],
                            x_t[:, col : col + PT],
                            start=True,
                            stop=False,
                        )
                        nc.tensor.matmul(
                            ps[:, o : o + PT],
                            w_t[:, N : 2 * N],
                            x_t[:, col + L : col + L + PT],
                            start=False,
                            stop=True,
                        )
                    # Alternate cast engines; odd blocks on scalar so each
                    # store (also on scalar) follows its last cast in
                    # program order on the same engine — no cross-engine
                    # sem hop on the final store.
                    if i % 2 == 1:
                        nc.scalar.copy(y_t[:, mb : mb + PB], ps[:])
                    else:
                        nc.vector.tensor_copy(y_t[:, mb : mb + PB], ps[:])
                    if mb + PB in stores:
                        so, se = prev_store, mb + PB
                        prev_store = se
                        nc.scalar.dma_start(
                            yT[:, mo + so : mo + se], y_t[:, so:se]
                        )

    _split_multi_waits(nc, mybir)
    return nc


_NC_CACHE = None


def _get_nc():
    global _NC_CACHE
    if _NC_CACHE is None:
        _NC_CACHE = _build_bass()
    return _NC_CACHE


def _run(in_maps, **kwargs):
    from concourse.bass_utils import run_bass_kernel_spmd

    return run_bass_kernel_spmd(_get_nc(), in_maps, list(range(G)), **kwargs)


def make_in_maps(x, w):
    x = np.asarray(x)
    w = np.asarray(w)
    in_maps = []
    for g in range(G):
        xg = x[g * MPC : (g + 1) * MPC, :]
        wg = w[g * N : (g + 1) * N, :]
        # Per chunk of L tokens starting at token T:
        #   xP[p, colbase + h*L + t] = xg.T[h*128+p, T+t]
        xgT = xg.T
        segs = []
        T = 0
        for chunks in TILE_CHUNKS:
            for L in chunks:
                seg = xgT[:, T : T + L].reshape(2, N, L)
                segs.append(seg.transpose(1, 0, 2).reshape(N, 2 * L))
                T += L
        xPg = np.ascontiguousarray(np.concatenate(segs, axis=1))
        # wP[p, h*128+n] = wg.T[h*128+p, n]
        wPg = np.ascontiguousarray(
            wg.T.reshape(2, N, N).transpose(1, 0, 2).reshape(N, K)
        )
        in_maps.append({"xP": xPg, "wP": wPg})
    return in_maps


def assemble(results, dtype):
    out = np.zeros((M, G * N), dtype=dtype)
    for g in range(G):
        yTg = np.asarray(results[g]["yT"])
        out[g * MPC : (g + 1) * MPC, g * N : (g + 1) * N] = yTg.T
    return out


def kernel(x, w):
    x = np.asarray(x)
    w = np.asarray(w)
    res = _run(make_in_maps(x, w))
    return assemble(res.results, x.dtype)
